# revision 3
# baseline (speedup 1.0000x reference)
"""Trainium2 Bass kernel for nn_BatchASTEncoder (batched AST / complete-binary-tree
GNN message passing).

Math (per batch column b):
    h[p] = W_c @ encodes[node_ids[p, b]] + b_c                    (1023 tree positions)
    for level d = 8..0:  h[parent] += W_sum @ (h[left] + h[right]) + 2*b_sum
    node_list = relu(h[POSTORDER]);  out2 = max_p node_list

Distribution: data-parallel over batch B=64 across 8 NeuronCores (8 columns
per core); encodes and the tiny weights are replicated.

Per-core dataflow (B_loc=8 batch columns, column index r = b*1023 + p):
  - h kept feature-major in SBUF: per (e-chunk, stage) tiles [128, 2046]
    (stage = 2 batch columns). Feature-major makes tree pair-sums strided
    free-dim vector ops and matmuls layout-preserving.
  - 4-stage pipeline: indirect-DMA row gather (the serial SWDGE descriptor
    generation is the pacing item) -> PE transpose-in -> W_c matmul (N=512
    groups) -> ACT bias-drain into h -> per-stage tree -> relu + store.
  - 2*W_sum_b is folded into the init bias of non-leaf positions.
  - Output is written feature-major; the host unshard step transposes to the
    reference row-major [P, B, E] layout and applies the postorder permute.
"""

import numpy as np

DEPTH = 10
P = 2**DEPTH - 1          # 1023
B = 64
E = 256
N_TOTAL = B * P           # 65472
N_CORES = 8
B_LOC = B // N_CORES      # 8
R = B_LOC * P             # 8184 columns per core
RP = 8192                 # gather padded to 64 blocks of 128 rows
NBLK = RP // 128          # 64
NSTAGE = 4
SB = 2                    # batch columns per stage
SW = SB * P               # stage width: 2046
BLK_PER_STAGE = NBLK // NSTAGE
GRP = 4                   # gather blocks per matmul group (N = 512)


def _postorder(p, out):
    if p >= P:
        return
    _postorder(2 * p + 1, out)
    _postorder(2 * p + 2, out)
    out.append(p)


_PO = []
_postorder(0, _PO)
POSTORDER = np.array(_PO, dtype=np.int32)

_NC_CACHE = [None]
LAST_RESULT = [None]
TRACE = [False]


def _drain_runs(g):
    """Column runs for matmul-group g (r in [512g, 512(g+1))), split at
    leaf/non-leaf and stage boundaries. Yields (r0, r1, leaf, stage)."""
    runs = []
    r = 512 * g
    end = min(512 * (g + 1), R)
    while r < end:
        b, p = divmod(r, P)
        leaf = p >= 511
        nxt = P * b + (P if leaf else 511)
        r1 = min(end, nxt)
        runs.append((r, r1, leaf, b // SB))
        r = r1
    return runs


def _build_nc():
    import concourse.bacc as bacc
    import concourse.mybir as mybir
    import concourse.tile as tile
    from concourse import bass
    from concourse.masks import make_identity

    f32 = mybir.dt.float32
    i32 = mybir.dt.int32
    AF = mybir.ActivationFunctionType
    AX = mybir.AxisListType

    nc = bacc.Bacc("TRN2", target_bir_lowering=False, debug=False)

    encodes = nc.dram_tensor("encodes", [N_TOTAL, E], f32, kind="ExternalInput")
    idx_d = nc.dram_tensor("idx", [128, NBLK], i32, kind="ExternalInput")
    wc_d = nc.dram_tensor("wc", [E, E], f32, kind="ExternalInput")
    ws_d = nc.dram_tensor("ws", [E, E], f32, kind="ExternalInput")
    bias_d = nc.dram_tensor("biases", [128, 4], f32, kind="ExternalInput")
    # feature-major output: [e_chunk, stage, 128, 2046]
    out_nl = nc.dram_tensor("out_nl", [2, NSTAGE, 128, SW], f32, kind="ExternalOutput")
    out_max = nc.dram_tensor("out_max", [128, 2 * B_LOC], f32, kind="ExternalOutput")

    with tile.TileContext(nc) as tc:
        with (
            tc.tile_pool(name="const", bufs=1) as cpool,
            tc.tile_pool(name="h", bufs=1) as hpool,
            tc.tile_pool(name="g", bufs=3) as gpool,
            tc.tile_pool(name="gtg", bufs=2) as gtgpool,
            tc.tile_pool(name="ks", bufs=2) as kspool,
            tc.tile_pool(name="ob", bufs=2) as obpool,
            tc.tile_pool(name="mx", bufs=1) as mxpool,
            tc.tile_pool(name="tp", bufs=2, space="PSUM") as tppool,
            tc.tile_pool(name="hp", bufs=2, space="PSUM") as hppool,
            tc.tile_pool(name="cs", bufs=1, space="PSUM") as cspool,
        ):
            wc0 = cpool.tile([128, E], f32)
            wc1 = cpool.tile([128, E], f32)
            ws0 = cpool.tile([128, E], f32)
            ws1 = cpool.tile([128, E], f32)
            bias = cpool.tile([128, 4], f32)
            idx = cpool.tile([128, NBLK], i32)
            ident = cpool.tile([128, 128], f32)
            nc.sync.dma_start(out=idx[:], in_=idx_d[:, :])
            nc.sync.dma_start(out=wc0[:], in_=wc_d[0:128, :])
            nc.sync.dma_start(out=wc1[:], in_=wc_d[128:256, :])
            nc.sync.dma_start(out=ws0[:], in_=ws_d[0:128, :])
            nc.sync.dma_start(out=ws1[:], in_=ws_d[128:256, :])
            nc.sync.dma_start(out=bias[:], in_=bias_d[:, :])
            make_identity(nc, ident[:])

            # h[e][s]: feature-major stage tiles
            hts = [
                [hpool.tile([128, SW], f32, name=f"h{e}s{s}") for s in range(NSTAGE)]
                for e in range(2)
            ]
            wcs = (wc0, wc1)
            wss = (ws0, ws1)

            mx = mxpool.tile([128, 2 * B_LOC], f32)

            for s in range(NSTAGE):
                # ---- init: gather + transpose-in + W_c + bias-drain -------
                for gi in range(GRP):
                    grp = s * GRP + gi
                    gtg = gtgpool.tile([128, GRP * E], f32)  # d-major: [d][blk][128]
                    for k in range(GRP):
                        blk = grp * GRP + k
                        g = gpool.tile([128, E], f32)
                        nc.gpsimd.indirect_dma_start(
                            out=g[:],
                            out_offset=None,
                            in_=encodes[:, :],
                            in_offset=bass.IndirectOffsetOnAxis(
                                ap=idx[:, blk : blk + 1], axis=0
                            ),
                        )
                        tp0 = tppool.tile([128, 128], f32, tag="tp")
                        tp1 = tppool.tile([128, 128], f32, tag="tp")
                        nc.tensor.transpose(
                            out=tp0[:], in_=g[:, 0:128], identity=ident[:]
                        )
                        nc.tensor.transpose(
                            out=tp1[:], in_=g[:, 128:256], identity=ident[:]
                        )
                        nc.vector.tensor_copy(
                            out=gtg[:, k * 128 : k * 128 + 128], in_=tp0[:]
                        )
                        nc.vector.tensor_copy(
                            out=gtg[:, 512 + k * 128 : 512 + k * 128 + 128],
                            in_=tp1[:],
                        )
                    for e in range(2):
                        es = slice(128 * e, 128 * e + 128)
                        hp = hppool.tile([128, 512], f32, tag=f"hp{e}")
                        nc.tensor.matmul(
                            out=hp[:], lhsT=wc0[:, es], rhs=gtg[:, 0:512],
                            start=True, stop=False,
                        )
                        nc.tensor.matmul(
                            out=hp[:], lhsT=wc1[:, es], rhs=gtg[:, 512:1024],
                            start=False, stop=True,
                        )
                        for r0, r1, leaf, rs in _drain_runs(grp):
                            bcol = e if leaf else 2 + e
                            l0 = r0 - SW * rs
                            nc.scalar.activation(
                                out=hts[e][rs][:, l0 : l0 + (r1 - r0)],
                                in_=hp[:, r0 - 512 * grp : r1 - 512 * grp],
                                func=AF.Identity,
                                bias=bias[:, bcol : bcol + 1],
                                scale=1.0,
                            )

                # ---- per-stage bottom-up tree ----------------------------
                hv = [
                    hts[e][s][:, :].rearrange("e (b q) -> e b q", b=SB)
                    for e in range(2)
                ]
                for d in range(DEPTH - 2, -1, -1):
                    p0 = 2**d - 1
                    n = 2**d
                    ks = [
                        kspool.tile([128, SB * n], f32, tag=f"ks{e}", name=f"ks{e}")
                        for e in range(2)
                    ]
                    for e in range(2):
                        kid2 = hv[e][:, :, 2 * p0 + 1 : 2 * p0 + 1 + 2 * n].rearrange(
                            "e b (n two) -> e b n two", two=2
                        )
                        nc.vector.tensor_add(
                            out=ks[e][:].rearrange("e (b n) -> e b n", b=SB),
                            in0=kid2[:, :, :, 0],
                            in1=kid2[:, :, :, 1],
                        )
                    for e in range(2):
                        es = slice(128 * e, 128 * e + 128)
                        cs = cspool.tile([128, SB * n], f32, tag=f"cs{e}", name=f"cs{e}")
                        nc.tensor.matmul(
                            out=cs[:], lhsT=ws0[:, es], rhs=ks[0][:, :],
                            start=True, stop=False,
                        )
                        nc.tensor.matmul(
                            out=cs[:], lhsT=ws1[:, es], rhs=ks[1][:, :],
                            start=False, stop=True,
                        )
                        nc.vector.tensor_add(
                            out=hv[e][:, :, p0 : p0 + n],
                            in0=hv[e][:, :, p0 : p0 + n],
                            in1=cs[:].rearrange("e (b n) -> e b n", b=SB),
                        )

                # ---- stage output: max, relu, store ----------------------
                for e in range(2):
                    mxp = mxpool.tile([128, SB], f32, tag=f"mxp{e}", name=f"mxp{e}")
                    nc.vector.tensor_reduce(
                        out=mxp[:], in_=hv[e][:, :, :], axis=AX.X,
                        op=mybir.AluOpType.max,
                    )
                    nc.scalar.activation(
                        out=mx[:, e * B_LOC + SB * s : e * B_LOC + SB * (s + 1)],
                        in_=mxp[:],
                        func=AF.Relu,
                    )
                    ob = obpool.tile([128, SW], f32)
                    nc.scalar.activation(out=ob[:], in_=hts[e][s][:, :], func=AF.Relu)
                    nc.sync.dma_start(out=out_nl[e, s, :, :], in_=ob[:])

            nc.sync.dma_start(out=out_max[:, :], in_=mx[:])

    nc.compile()
    return nc


def kernel(**inputs):
    from concourse.bass_utils import run_bass_kernel_spmd

    encodes = np.ascontiguousarray(np.asarray(inputs["encodes"], dtype=np.float32))
    node_ids = np.asarray(inputs["node_ids"])
    wc = np.ascontiguousarray(np.asarray(inputs["W_c_w"], dtype=np.float32).T)
    ws = np.ascontiguousarray(np.asarray(inputs["W_sum_w"], dtype=np.float32).T)
    bc = np.asarray(inputs["W_c_b"], dtype=np.float32)
    bs = np.asarray(inputs["W_sum_b"], dtype=np.float32)

    bias = np.zeros((128, 4), np.float32)
    bias[:, 0] = bc[0:128]
    bias[:, 1] = bc[128:256]
    bias[:, 2] = bc[0:128] + 2.0 * bs[0:128]
    bias[:, 3] = bc[128:256] + 2.0 * bs[128:256]

    in_maps = []
    for c in range(N_CORES):
        nid = np.asarray(node_ids[:, c * B_LOC : (c + 1) * B_LOC], dtype=np.int32)
        flat = np.zeros(RP, np.int32)
        flat[:R] = nid.T.reshape(-1)  # r = b*1023 + p
        idx = np.ascontiguousarray(flat.reshape(NBLK, 128).T)  # [part, blk]
        in_maps.append(
            {"encodes": encodes, "idx": idx, "wc": wc, "ws": ws, "biases": bias}
        )

    if _NC_CACHE[0] is None:
        _NC_CACHE[0] = _build_nc()
    nc = _NC_CACHE[0]

    res = run_bass_kernel_spmd(
        nc, in_maps, core_ids=list(range(N_CORES)), trace=TRACE[0]
    )
    LAST_RESULT[0] = res

    node_list = np.empty((P, B, E), np.float32)
    mx = np.empty((B, E), np.float32)
    for c in range(N_CORES):
        r = res.results[c]
        fm = r["out_nl"]  # [2, 4, 128, 2046]
        # -> [b_loc, p, e]: value(e, b) = fm[e//128, b//2, e%128, (b%2)*1023 + p]
        nl = (
            fm.reshape(2, NSTAGE, 128, SB, P)
            .transpose(1, 3, 4, 0, 2)  # [s, bb, p, e_chunk, part]
            .reshape(B_LOC, P, E)
        )
        node_list[:, c * B_LOC : (c + 1) * B_LOC, :] = nl.transpose(1, 0, 2)[POSTORDER]
        om = r["out_max"]  # [128, 16]
        mx[c * B_LOC : (c + 1) * B_LOC, 0:128] = om[:, 0:B_LOC].T
        mx[c * B_LOC : (c + 1) * B_LOC, 128:256] = om[:, B_LOC : 2 * B_LOC].T
    return node_list, mx


# revision 4
# speedup vs baseline: 1.2288x; 1.2288x over previous
"""Trainium2 Bass kernel for nn_BatchASTEncoder (batched AST / complete-binary-tree
GNN message passing).

Math (per batch column b):
    h[p] = W_c @ encodes[node_ids[p, b]] + b_c                    (1023 tree positions)
    for level d = 8..0:  h[parent] += W_sum @ (h[left] + h[right]) + 2*b_sum
    node_list = relu(h[POSTORDER]);  out2 = max_p node_list

Distribution: data-parallel over batch B=64 across 8 NeuronCores (8 columns
per core); encodes and the tiny weights are replicated.

Per-core dataflow (B_loc=8 batch columns, column index r = b*1023 + p):
  - h kept feature-major in SBUF: per (e-chunk, stage) tiles [128, 2046]
    (stage = 2 batch columns). Feature-major makes tree pair-sums strided
    free-dim vector ops and matmuls layout-preserving.
  - 4-stage pipeline: indirect-DMA row gather (the serial SWDGE descriptor
    generation is the pacing item) -> PE transpose-in -> W_c matmul (N=512
    groups) -> ACT bias-drain into h -> per-stage tree -> relu + store.
  - 2*W_sum_b is folded into the init bias of non-leaf positions.
  - Output is written feature-major; the host unshard step transposes to the
    reference row-major [P, B, E] layout and applies the postorder permute.
"""

import numpy as np

DEPTH = 10
P = 2**DEPTH - 1          # 1023
B = 64
E = 256
N_TOTAL = B * P           # 65472
N_CORES = 8
B_LOC = B // N_CORES      # 8
R = B_LOC * P             # 8184 columns per core
RP = 8192                 # gather padded to 64 blocks of 128 rows
NBLK = RP // 128          # 64
NSTAGE = 4
SB = 2                    # batch columns per stage
SW = SB * P               # stage width: 2046
BLK_PER_STAGE = NBLK // NSTAGE
GRP = 4                   # gather blocks per matmul group (N = 512)


def _postorder(p, out):
    if p >= P:
        return
    _postorder(2 * p + 1, out)
    _postorder(2 * p + 2, out)
    out.append(p)


_PO = []
_postorder(0, _PO)
POSTORDER = np.array(_PO, dtype=np.int32)

_NC_CACHE = [None]
LAST_RESULT = [None]
TRACE = [False]


def _drain_runs(g):
    """Column runs for matmul-group g (r in [512g, 512(g+1))), split at
    leaf/non-leaf and stage boundaries. Yields (r0, r1, leaf, stage)."""
    runs = []
    r = 512 * g
    end = min(512 * (g + 1), R)
    while r < end:
        b, p = divmod(r, P)
        leaf = p >= 511
        nxt = P * b + (P if leaf else 511)
        r1 = min(end, nxt)
        runs.append((r, r1, leaf, b // SB))
        r = r1
    return runs


def _build_nc():
    import concourse.bacc as bacc
    import concourse.mybir as mybir
    import concourse.tile as tile
    from concourse import bass
    from concourse.masks import make_identity

    f32 = mybir.dt.float32
    i32 = mybir.dt.int32
    AF = mybir.ActivationFunctionType
    AX = mybir.AxisListType

    nc = bacc.Bacc("TRN2", target_bir_lowering=False, debug=False)

    encodes = nc.dram_tensor("encodes", [N_TOTAL, E], f32, kind="ExternalInput")
    idx_d = nc.dram_tensor("idx", [128, NBLK], i32, kind="ExternalInput")
    wc_d = nc.dram_tensor("wc", [E, E], f32, kind="ExternalInput")
    ws_d = nc.dram_tensor("ws", [E, E], f32, kind="ExternalInput")
    bias_d = nc.dram_tensor("biases", [128, 4], f32, kind="ExternalInput")
    # feature-major output: [e_chunk, stage, 128, 2046]
    out_nl = nc.dram_tensor("out_nl", [2, NSTAGE, 128, SW], f32, kind="ExternalOutput")
    out_max = nc.dram_tensor("out_max", [128, 2 * B_LOC], f32, kind="ExternalOutput")

    with tile.TileContext(nc) as tc:
        with (
            tc.tile_pool(name="const", bufs=1) as cpool,
            tc.tile_pool(name="h", bufs=1) as hpool,
            tc.tile_pool(name="g", bufs=6) as gpool,
            tc.tile_pool(name="gtg", bufs=2) as gtgpool,
            tc.tile_pool(name="ks", bufs=2) as kspool,
            tc.tile_pool(name="ob", bufs=2) as obpool,
            tc.tile_pool(name="mx", bufs=1) as mxpool,
            tc.tile_pool(name="tp", bufs=4, space="PSUM") as tppool,
            tc.tile_pool(name="hp", bufs=2, space="PSUM") as hppool,
        ):
            wc0 = cpool.tile([128, E], f32)
            wc1 = cpool.tile([128, E], f32)
            ws0 = cpool.tile([128, E], f32)
            ws1 = cpool.tile([128, E], f32)
            bias = cpool.tile([128, 4], f32)
            idx = cpool.tile([128, NBLK], i32)
            ident = cpool.tile([128, 128], f32)
            nc.sync.dma_start(out=idx[:], in_=idx_d[:, :])
            nc.sync.dma_start(out=wc0[:], in_=wc_d[0:128, :])
            nc.sync.dma_start(out=wc1[:], in_=wc_d[128:256, :])
            nc.sync.dma_start(out=ws0[:], in_=ws_d[0:128, :])
            nc.sync.dma_start(out=ws1[:], in_=ws_d[128:256, :])
            nc.sync.dma_start(out=bias[:], in_=bias_d[:, :])
            make_identity(nc, ident[:])

            # h[e][s]: feature-major stage tiles
            hts = [
                [hpool.tile([128, SW], f32, name=f"h{e}s{s}") for s in range(NSTAGE)]
                for e in range(2)
            ]
            wcs = (wc0, wc1)
            wss = (ws0, ws1)

            mx = mxpool.tile([128, 2 * B_LOC], f32)

            for s in range(NSTAGE):
                # ---- init: gather + transpose-in + W_c + bias-drain -------
                for gi in range(GRP):
                    grp = s * GRP + gi
                    gtg = gtgpool.tile([128, GRP * E], f32)  # d-major: [d][blk][128]
                    for k in range(GRP):
                        blk = grp * GRP + k
                        g = gpool.tile([128, E], f32)
                        nc.gpsimd.indirect_dma_start(
                            out=g[:],
                            out_offset=None,
                            in_=encodes[:, :],
                            in_offset=bass.IndirectOffsetOnAxis(
                                ap=idx[:, blk : blk + 1], axis=0
                            ),
                        )
                        tp0 = tppool.tile([128, 128], f32, tag="tp")
                        tp1 = tppool.tile([128, 128], f32, tag="tp")
                        nc.tensor.transpose(
                            out=tp0[:], in_=g[:, 0:128], identity=ident[:]
                        )
                        nc.tensor.transpose(
                            out=tp1[:], in_=g[:, 128:256], identity=ident[:]
                        )
                        nc.vector.tensor_copy(
                            out=gtg[:, k * 128 : k * 128 + 128], in_=tp0[:]
                        )
                        nc.scalar.copy(
                            out=gtg[:, 512 + k * 128 : 512 + k * 128 + 128],
                            in_=tp1[:],
                        )
                    for e in range(2):
                        es = slice(128 * e, 128 * e + 128)
                        hp = hppool.tile([128, 512], f32, tag=f"hp{e}")
                        nc.tensor.matmul(
                            out=hp[:], lhsT=wc0[:, es], rhs=gtg[:, 0:512],
                            start=True, stop=False,
                        )
                        nc.tensor.matmul(
                            out=hp[:], lhsT=wc1[:, es], rhs=gtg[:, 512:1024],
                            start=False, stop=True,
                        )
                        for r0, r1, leaf, rs in _drain_runs(grp):
                            bcol = e if leaf else 2 + e
                            l0 = r0 - SW * rs
                            nc.scalar.activation(
                                out=hts[e][rs][:, l0 : l0 + (r1 - r0)],
                                in_=hp[:, r0 - 512 * grp : r1 - 512 * grp],
                                func=AF.Identity,
                                bias=bias[:, bcol : bcol + 1],
                                scale=1.0,
                            )

                # ---- per-stage bottom-up tree ----------------------------
                hv = [
                    hts[e][s][:, :].rearrange("e (b q) -> e b q", b=SB)
                    for e in range(2)
                ]
                for d in range(DEPTH - 2, -1, -1):
                    p0 = 2**d - 1
                    n = 2**d
                    ks = [
                        kspool.tile([128, SB * n], f32, tag=f"ks{e}", name=f"ks{e}")
                        for e in range(2)
                    ]
                    for e in range(2):
                        kid2 = hv[e][:, :, 2 * p0 + 1 : 2 * p0 + 1 + 2 * n].rearrange(
                            "e b (n two) -> e b n two", two=2
                        )
                        nc.vector.tensor_add(
                            out=ks[e][:].rearrange("e (b n) -> e b n", b=SB),
                            in0=kid2[:, :, :, 0],
                            in1=kid2[:, :, :, 1],
                        )
                    for e in range(2):
                        es = slice(128 * e, 128 * e + 128)
                        cs = hppool.tile([128, 512], f32, tag=f"hp{e}", name=f"cs{e}")[:, : SB * n]
                        nc.tensor.matmul(
                            out=cs[:], lhsT=ws0[:, es], rhs=ks[0][:, :],
                            start=True, stop=False,
                        )
                        nc.tensor.matmul(
                            out=cs[:], lhsT=ws1[:, es], rhs=ks[1][:, :],
                            start=False, stop=True,
                        )
                        nc.vector.tensor_add(
                            out=hv[e][:, :, p0 : p0 + n],
                            in0=hv[e][:, :, p0 : p0 + n],
                            in1=cs[:].rearrange("e (b n) -> e b n", b=SB),
                        )

                # ---- stage output: max, relu, store ----------------------
                for e in range(2):
                    mxp = mxpool.tile([128, SB], f32, tag=f"mxp{e}", name=f"mxp{e}")
                    nc.vector.tensor_reduce(
                        out=mxp[:], in_=hv[e][:, :, :], axis=AX.X,
                        op=mybir.AluOpType.max,
                    )
                    nc.scalar.activation(
                        out=mx[:, e * B_LOC + SB * s : e * B_LOC + SB * (s + 1)],
                        in_=mxp[:],
                        func=AF.Relu,
                    )
                    ob = obpool.tile([128, SW], f32)
                    nc.scalar.activation(out=ob[:], in_=hts[e][s][:, :], func=AF.Relu)
                    nc.sync.dma_start(out=out_nl[e, s, :, :], in_=ob[:])

            nc.sync.dma_start(out=out_max[:, :], in_=mx[:])

    nc.compile()
    return nc


def kernel(**inputs):
    from concourse.bass_utils import run_bass_kernel_spmd

    encodes = np.ascontiguousarray(np.asarray(inputs["encodes"], dtype=np.float32))
    node_ids = np.asarray(inputs["node_ids"])
    wc = np.ascontiguousarray(np.asarray(inputs["W_c_w"], dtype=np.float32).T)
    ws = np.ascontiguousarray(np.asarray(inputs["W_sum_w"], dtype=np.float32).T)
    bc = np.asarray(inputs["W_c_b"], dtype=np.float32)
    bs = np.asarray(inputs["W_sum_b"], dtype=np.float32)

    bias = np.zeros((128, 4), np.float32)
    bias[:, 0] = bc[0:128]
    bias[:, 1] = bc[128:256]
    bias[:, 2] = bc[0:128] + 2.0 * bs[0:128]
    bias[:, 3] = bc[128:256] + 2.0 * bs[128:256]

    in_maps = []
    for c in range(N_CORES):
        nid = np.asarray(node_ids[:, c * B_LOC : (c + 1) * B_LOC], dtype=np.int32)
        flat = np.zeros(RP, np.int32)
        flat[:R] = nid.T.reshape(-1)  # r = b*1023 + p
        idx = np.ascontiguousarray(flat.reshape(NBLK, 128).T)  # [part, blk]
        in_maps.append(
            {"encodes": encodes, "idx": idx, "wc": wc, "ws": ws, "biases": bias}
        )

    if _NC_CACHE[0] is None:
        _NC_CACHE[0] = _build_nc()
    nc = _NC_CACHE[0]

    res = run_bass_kernel_spmd(
        nc, in_maps, core_ids=list(range(N_CORES)), trace=TRACE[0]
    )
    LAST_RESULT[0] = res

    node_list = np.empty((P, B, E), np.float32)
    mx = np.empty((B, E), np.float32)
    for c in range(N_CORES):
        r = res.results[c]
        fm = r["out_nl"]  # [2, 4, 128, 2046]
        # -> [b_loc, p, e]: value(e, b) = fm[e//128, b//2, e%128, (b%2)*1023 + p]
        nl = (
            fm.reshape(2, NSTAGE, 128, SB, P)
            .transpose(1, 3, 4, 0, 2)  # [s, bb, p, e_chunk, part]
            .reshape(B_LOC, P, E)
        )
        node_list[:, c * B_LOC : (c + 1) * B_LOC, :] = nl.transpose(1, 0, 2)[POSTORDER]
        om = r["out_max"]  # [128, 16]
        mx[c * B_LOC : (c + 1) * B_LOC, 0:128] = om[:, 0:B_LOC].T
        mx[c * B_LOC : (c + 1) * B_LOC, 128:256] = om[:, B_LOC : 2 * B_LOC].T
    return node_list, mx


# revision 7
# speedup vs baseline: 1.3914x; 1.1323x over previous
"""Trainium2 Bass kernel for nn_BatchASTEncoder (batched AST / complete-binary-tree
GNN message passing).

Math (per batch column b):
    h[p] = W_c @ encodes[node_ids[p, b]] + b_c                    (1023 tree positions)
    for level d = 8..0:  h[parent] += W_sum @ (h[left] + h[right]) + 2*b_sum
    node_list = relu(h[POSTORDER]);  out2 = max_p node_list

Distribution: data-parallel over batch B=64 across 8 NeuronCores (8 columns
per core); encodes and the tiny weights are replicated.

Per-core dataflow (B_loc=8 batch columns, column index r = b*1023 + p):
  - h kept feature-major in SBUF: per (e-chunk, stage) tiles [128, 2046]
    (stage = 2 batch columns). Feature-major makes tree pair-sums strided
    free-dim vector ops and matmuls layout-preserving.
  - 4-stage pipeline: row gather (SWDGE descriptor generation is the pacing
    item) -> PE transpose-in -> W_c matmul (N=512 groups) -> ACT bias-drain
    into h -> per-stage tree -> relu + store.
  - Gather: one dma_gather per batch column. dma_gather indices are int16,
    so in_ap is based at the table midpoint and indices are signed offsets
    (idx - 32768); 17 trailing dummy indices keep the ext-isa kernel's
    trailing-negative truncation away from real rows.
  - Matmuls run in float32r (single-pass fp32, ~1e-4 relative error) instead
    of float32 (two-pass); producers of matmul operands write f32r tiles.
  - 2*W_sum_b is folded into the init bias of non-leaf positions.
  - Output is written feature-major; the host unshard step transposes to the
    reference row-major [P, B, E] layout and applies the postorder permute.
"""

import numpy as np

DEPTH = 10
P = 2**DEPTH - 1          # 1023
B = 64
E = 256
N_TOTAL = B * P           # 65472
HALF = 32768              # dma_gather int16 base offset
N_CORES = 8
B_LOC = B // N_CORES      # 8
R = B_LOC * P             # 8184 columns per core
NSTAGE = 4
SB = 2                    # batch columns per stage
SW = SB * P               # stage width: 2046
NI = 1040                 # dma_gather indices per call: 1023 real + 17 dummies
NIB = NI // 128 + 1       # dst j-blocks per call (9)
GATHER = "dma_gather"     # or "indirect"
USE_F32R = True


def _postorder(p, out):
    if p >= P:
        return
    _postorder(2 * p + 1, out)
    _postorder(2 * p + 2, out)
    out.append(p)


_PO = []
_postorder(0, _PO)
POSTORDER = np.array(_PO, dtype=np.int32)

_NC_CACHE = [None]
LAST_RESULT = [None]
TRACE = [False]


def _drain_runs_b(grp_local):
    """Column runs for per-b matmul group (local p in [512*g, 512*(g+1)) cap 1023),
    split at the leaf boundary p=511. Yields (p0, p1, leaf)."""
    runs = []
    p = 512 * grp_local
    end = min(512 * (grp_local + 1), P)
    while p < end:
        leaf = p >= 511
        nxt = P if leaf else 511
        p1 = min(end, nxt)
        runs.append((p, p1, leaf))
        p = p1
    return runs


def _build_nc():
    import concourse.bacc as bacc
    import concourse.mybir as mybir
    import concourse.tile as tile
    from concourse import bass
    from concourse.masks import make_identity

    f32 = mybir.dt.float32
    f32m = mybir.dt.float32r if USE_F32R else f32
    i32 = mybir.dt.int32
    i16 = mybir.dt.int16
    AF = mybir.ActivationFunctionType
    AX = mybir.AxisListType

    nc = bacc.Bacc("TRN2", target_bir_lowering=False, debug=False)

    encodes = nc.dram_tensor("encodes", [N_TOTAL, E], f32, kind="ExternalInput")
    if GATHER == "dma_gather":
        idx_d = nc.dram_tensor(
            "idx", [128, B_LOC * (NI // 16)], i16, kind="ExternalInput"
        )
    else:
        idx_d = nc.dram_tensor("idx", [128, 64], i32, kind="ExternalInput")
    wc_d = nc.dram_tensor("wc", [E, E], f32, kind="ExternalInput")
    ws_d = nc.dram_tensor("ws", [E, E], f32, kind="ExternalInput")
    bias_d = nc.dram_tensor("biases", [128, 4], f32, kind="ExternalInput")
    out_nl = nc.dram_tensor("out_nl", [2, NSTAGE, 128, SW], f32, kind="ExternalOutput")
    out_max = nc.dram_tensor("out_max", [128, 2 * B_LOC], f32, kind="ExternalOutput")

    with tile.TileContext(nc) as tc:
        with (
            tc.tile_pool(name="const", bufs=1) as cpool,
            tc.tile_pool(name="h", bufs=1) as hpool,
            tc.tile_pool(name="g", bufs=3) as gpool,
            tc.tile_pool(name="gtg", bufs=3) as gtgpool,
            tc.tile_pool(name="ks", bufs=2) as kspool,
            tc.tile_pool(name="ob", bufs=2) as obpool,
            tc.tile_pool(name="mx", bufs=1) as mxpool,
            tc.tile_pool(name="tp", bufs=4, space="PSUM") as tppool,
            tc.tile_pool(name="hp", bufs=2, space="PSUM") as hppool,
        ):
            wc_f = cpool.tile([128, 2 * E], f32)
            ws_f = cpool.tile([128, 2 * E], f32)
            wc = cpool.tile([128, 2 * E], f32m)   # [d-chunk part, dchunk*256 + e]
            ws = cpool.tile([128, 2 * E], f32m)
            bias = cpool.tile([128, 4], f32)
            if GATHER == "dma_gather":
                idx = cpool.tile([128, B_LOC * (NI // 16)], i16)
            else:
                idx = cpool.tile([128, 64], i32)
            ident = cpool.tile([128, 128], f32)
            nc.sync.dma_start(out=idx[:], in_=idx_d[:, :])
            nc.sync.dma_start(out=wc_f[:, 0:256], in_=wc_d[0:128, :])
            nc.sync.dma_start(out=wc_f[:, 256:512], in_=wc_d[128:256, :])
            nc.sync.dma_start(out=ws_f[:, 0:256], in_=ws_d[0:128, :])
            nc.sync.dma_start(out=ws_f[:, 256:512], in_=ws_d[128:256, :])
            nc.sync.dma_start(out=bias[:], in_=bias_d[:, :])
            nc.vector.tensor_copy(out=wc[:], in_=wc_f[:])
            nc.vector.tensor_copy(out=ws[:], in_=ws_f[:])
            make_identity(nc, ident[:])

            hts = [
                [hpool.tile([128, SW], f32, name=f"h{e}s{s}") for s in range(NSTAGE)]
                for e in range(2)
            ]
            mx = mxpool.tile([128, 2 * B_LOC], f32)

            for s in range(NSTAGE):
                for bb in range(SB):
                    b = SB * s + bb
                    # ---- gather one batch column (1023 rows) -------------
                    g = gpool.tile([128, NIB * E], f32)
                    if GATHER == "dma_gather":
                        nc.gpsimd.dma_gather(
                            out_ap=g[:].rearrange("p (j e) -> p j e", e=E),
                            in_ap=encodes[HALF:, :],
                            idxs_ap=idx[:, b * (NI // 16) : (b + 1) * (NI // 16)],
                            num_idxs=NI,
                            num_idxs_reg=NI,
                            elem_size=E,
                            single_packet=False,
                        )
                    else:
                        for j in range(8):
                            nc.gpsimd.indirect_dma_start(
                                out=g[:, j * E : (j + 1) * E],
                                out_offset=None,
                                in_=encodes[:, :],
                                in_offset=bass.IndirectOffsetOnAxis(
                                    ap=idx[:, b * 8 + j : b * 8 + j + 1], axis=0
                                ),
                            )
                    # ---- transpose-in + W_c + bias-drain -----------------
                    for gi in range(2):  # local groups of 4 blocks (N=512)
                        gtg = gtgpool.tile([128, 4 * E], f32m)  # d-major
                        for k in range(4):
                            j = gi * 4 + k
                            tp0 = tppool.tile([128, 128], f32, tag="tp")
                            tp1 = tppool.tile([128, 128], f32, tag="tp")
                            nc.tensor.transpose(
                                out=tp0[:], in_=g[:, j * E : j * E + 128],
                                identity=ident[:],
                            )
                            nc.tensor.transpose(
                                out=tp1[:], in_=g[:, j * E + 128 : (j + 1) * E],
                                identity=ident[:],
                            )
                            nc.vector.tensor_copy(
                                out=gtg[:, k * 128 : k * 128 + 128], in_=tp0[:]
                            )
                            nc.scalar.copy(
                                out=gtg[:, 512 + k * 128 : 512 + k * 128 + 128],
                                in_=tp1[:],
                            )
                        for e in range(2):
                            hp = hppool.tile([128, 512], f32, tag=f"hp{e}")
                            nc.tensor.matmul(
                                out=hp[:],
                                lhsT=wc[:, 128 * e : 128 * e + 128],
                                rhs=gtg[:, 0:512],
                                start=True, stop=False,
                            )
                            nc.tensor.matmul(
                                out=hp[:],
                                lhsT=wc[:, 256 + 128 * e : 256 + 128 * e + 128],
                                rhs=gtg[:, 512:1024],
                                start=False, stop=True,
                            )
                            for p0, p1, leaf in _drain_runs_b(gi):
                                bcol = e if leaf else 2 + e
                                nc.scalar.activation(
                                    out=hts[e][s][
                                        :, bb * P + p0 : bb * P + p1
                                    ],
                                    in_=hp[:, p0 - 512 * gi : p1 - 512 * gi],
                                    func=AF.Identity,
                                    bias=bias[:, bcol : bcol + 1],
                                    scale=1.0,
                                )

                # ---- per-stage bottom-up tree ----------------------------
                hv = [
                    hts[e][s][:, :].rearrange("e (b q) -> e b q", b=SB)
                    for e in range(2)
                ]
                for d in range(DEPTH - 2, -1, -1):
                    p0 = 2**d - 1
                    n = 2**d
                    ks = [
                        kspool.tile([128, SB * n], f32m, tag=f"ks{e}", name=f"ks{e}")
                        for e in range(2)
                    ]
                    for e in range(2):
                        kid2 = hv[e][:, :, 2 * p0 + 1 : 2 * p0 + 1 + 2 * n].rearrange(
                            "e b (n two) -> e b n two", two=2
                        )
                        nc.vector.tensor_add(
                            out=ks[e][:].rearrange("e (b n) -> e b n", b=SB),
                            in0=kid2[:, :, :, 0],
                            in1=kid2[:, :, :, 1],
                        )
                    for e in range(2):
                        cs = hppool.tile(
                            [128, 512], f32, tag=f"hp{e}", name=f"cs{e}"
                        )[:, : SB * n]
                        nc.tensor.matmul(
                            out=cs[:], lhsT=ws[:, 128 * e : 128 * e + 128],
                            rhs=ks[0][:, :],
                            start=True, stop=False,
                        )
                        nc.tensor.matmul(
                            out=cs[:], lhsT=ws[:, 256 + 128 * e : 256 + 128 * e + 128],
                            rhs=ks[1][:, :],
                            start=False, stop=True,
                        )
                        nc.vector.tensor_add(
                            out=hv[e][:, :, p0 : p0 + n],
                            in0=hv[e][:, :, p0 : p0 + n],
                            in1=cs[:].rearrange("e (b n) -> e b n", b=SB),
                        )

                # ---- stage output: max, relu, store ----------------------
                for e in range(2):
                    mxp = mxpool.tile([128, SB], f32, tag=f"mxp{e}", name=f"mxp{e}")
                    nc.vector.tensor_reduce(
                        out=mxp[:], in_=hv[e][:, :, :], axis=AX.X,
                        op=mybir.AluOpType.max,
                    )
                    nc.scalar.activation(
                        out=mx[:, e * B_LOC + SB * s : e * B_LOC + SB * (s + 1)],
                        in_=mxp[:],
                        func=AF.Relu,
                    )
                    ob = obpool.tile([128, SW], f32)
                    nc.scalar.activation(out=ob[:], in_=hts[e][s][:, :], func=AF.Relu)
                    nc.sync.dma_start(out=out_nl[e, s, :, :], in_=ob[:])

            nc.sync.dma_start(out=out_max[:, :], in_=mx[:])

    nc.compile()
    return nc


def kernel(**inputs):
    from concourse.bass_utils import run_bass_kernel_spmd

    encodes = np.ascontiguousarray(np.asarray(inputs["encodes"], dtype=np.float32))
    node_ids = np.asarray(inputs["node_ids"])
    wc = np.ascontiguousarray(np.asarray(inputs["W_c_w"], dtype=np.float32).T)
    ws = np.ascontiguousarray(np.asarray(inputs["W_sum_w"], dtype=np.float32).T)
    bc = np.asarray(inputs["W_c_b"], dtype=np.float32)
    bs = np.asarray(inputs["W_sum_b"], dtype=np.float32)

    bias = np.zeros((128, 4), np.float32)
    bias[:, 0] = bc[0:128]
    bias[:, 1] = bc[128:256]
    bias[:, 2] = bc[0:128] + 2.0 * bs[0:128]
    bias[:, 3] = bc[128:256] + 2.0 * bs[128:256]

    in_maps = []
    for c in range(N_CORES):
        nid = np.asarray(node_ids[:, c * B_LOC : (c + 1) * B_LOC], dtype=np.int64)
        if GATHER == "dma_gather":
            cols = []
            for b in range(B_LOC):
                arr = np.zeros(NI, np.int64)
                arr[:P] = nid[:, b] - HALF
                arr[P:] = 0  # dummy: offset 0 (row HALF), keeps trailing idx >= 0
                arr16 = arr.astype(np.int16)
                wrapped = arr16.reshape(NI // 16, 16).T  # [16, NI/16]
                cols.append(np.tile(wrapped, (8, 1)))
            idx = np.ascontiguousarray(np.concatenate(cols, axis=1))
        else:
            idx = np.zeros((128, 64), np.int32)
            for b in range(B_LOC):
                col = np.zeros(1024, np.int64)
                col[:P] = nid[:, b]
                idx[:, b * 8 : (b + 1) * 8] = col.reshape(8, 128).T
            idx = np.ascontiguousarray(idx)
        in_maps.append(
            {"encodes": encodes, "idx": idx, "wc": wc, "ws": ws, "biases": bias}
        )

    if _NC_CACHE[0] is None:
        _NC_CACHE[0] = _build_nc()
    nc = _NC_CACHE[0]

    res = run_bass_kernel_spmd(
        nc, in_maps, core_ids=list(range(N_CORES)), trace=TRACE[0]
    )
    LAST_RESULT[0] = res

    node_list = np.empty((P, B, E), np.float32)
    mx = np.empty((B, E), np.float32)
    for c in range(N_CORES):
        r = res.results[c]
        fm = r["out_nl"]  # [2, 4, 128, 2046]
        nl = (
            fm.reshape(2, NSTAGE, 128, SB, P)
            .transpose(1, 3, 4, 0, 2)  # [s, bb, p, e_chunk, part]
            .reshape(B_LOC, P, E)
        )
        node_list[:, c * B_LOC : (c + 1) * B_LOC, :] = nl.transpose(1, 0, 2)[POSTORDER]
        om = r["out_max"]
        mx[c * B_LOC : (c + 1) * B_LOC, 0:128] = om[:, 0:B_LOC].T
        mx[c * B_LOC : (c + 1) * B_LOC, 128:256] = om[:, B_LOC : 2 * B_LOC].T
    return node_list, mx


# revision 8
# speedup vs baseline: 1.4355x; 1.0317x over previous
"""Trainium2 Bass kernel for nn_BatchASTEncoder (batched AST / complete-binary-tree
GNN message passing).

Math (per batch column b):
    h[p] = W_c @ encodes[node_ids[p, b]] + b_c                    (1023 tree positions)
    for level d = 8..0:  h[parent] += W_sum @ (h[left] + h[right]) + 2*b_sum
    node_list = relu(h[POSTORDER]);  out2 = max_p node_list

Distribution: data-parallel over batch B=64 across 8 NeuronCores (8 columns
per core); encodes and the tiny weights are replicated.

Per-core dataflow (B_loc=8 batch columns, column index r = b*1023 + p):
  - h kept feature-major in SBUF: per (e-chunk, stage) tiles [128, 2046]
    (stage = 2 batch columns). Feature-major makes tree pair-sums strided
    free-dim vector ops and matmuls layout-preserving.
  - Software-pipelined stages with a one-stage skew in emission order so each
    engine's FIFO interleaves stage s's tree/output with stage s+1's init;
    the serial SWDGE gather descriptor generation is the pacing item.
  - Gather: one dma_gather per batch column. dma_gather indices are int16,
    so in_ap is based at the table midpoint and indices are signed offsets
    (idx - 32768); 17 trailing dummy indices keep the ext-isa kernel's
    trailing-negative truncation away from real rows.
  - Matmuls run in float32r (single-pass fp32, ~1e-4 relative error) instead
    of float32 (two-pass); producers of matmul operands write f32r tiles.
  - 2*W_sum_b is folded into the init bias of non-leaf positions.
  - Output is written feature-major; the host unshard step transposes to the
    reference row-major [P, B, E] layout and applies the postorder permute.
"""

import numpy as np

DEPTH = 10
P = 2**DEPTH - 1          # 1023
B = 64
E = 256
N_TOTAL = B * P           # 65472
HALF = 32768              # dma_gather int16 base offset
N_CORES = 8
B_LOC = B // N_CORES      # 8
R = B_LOC * P             # 8184 columns per core
NSTAGE = 4
SB = 2                    # batch columns per stage
SW = SB * P               # stage width: 2046
NI = 1040                 # dma_gather indices per call: 1023 real + 17 dummies
NIB = NI // 128 + 1       # dst j-blocks per call (9)
CPACK = 1156              # packed f32 consts: wc(512) ws(512) bias(4) ident(128)
GATHER = "dma_gather"     # or "indirect"
USE_F32R = True


def _postorder(p, out):
    if p >= P:
        return
    _postorder(2 * p + 1, out)
    _postorder(2 * p + 2, out)
    out.append(p)


_PO = []
_postorder(0, _PO)
POSTORDER = np.array(_PO, dtype=np.int32)

_NC_CACHE = [None]
LAST_RESULT = [None]
TRACE = [False]


def _drain_runs_b(grp_local):
    """Column runs for per-b matmul group (local p in [512g, 512(g+1)) cap 1023),
    split at the leaf boundary p=511. Yields (p0, p1, leaf)."""
    runs = []
    p = 512 * grp_local
    end = min(512 * (grp_local + 1), P)
    while p < end:
        leaf = p >= 511
        nxt = P if leaf else 511
        p1 = min(end, nxt)
        runs.append((p, p1, leaf))
        p = p1
    return runs


def _build_nc():
    import concourse.bacc as bacc
    import concourse.mybir as mybir
    import concourse.tile as tile
    from concourse import bass
    from concourse.masks import make_identity

    f32 = mybir.dt.float32
    f32m = mybir.dt.float32r if USE_F32R else f32
    i16 = mybir.dt.int16
    i32 = mybir.dt.int32
    AF = mybir.ActivationFunctionType
    AX = mybir.AxisListType

    nc = bacc.Bacc("TRN2", target_bir_lowering=False, debug=False)

    encodes = nc.dram_tensor("encodes", [N_TOTAL, E], f32, kind="ExternalInput")
    if GATHER == "dma_gather":
        idx_d = nc.dram_tensor(
            "idx", [128, B_LOC * (NI // 16)], i16, kind="ExternalInput"
        )
    else:
        idx_d = nc.dram_tensor("idx", [128, 64], i32, kind="ExternalInput")
    cpk_d = nc.dram_tensor("cpack", [128, CPACK], f32, kind="ExternalInput")
    out_nl = nc.dram_tensor("out_nl", [2, NSTAGE, 128, SW], f32, kind="ExternalOutput")
    out_max = nc.dram_tensor("out_max", [128, 2 * B_LOC], f32, kind="ExternalOutput")

    with tile.TileContext(nc) as tc:
        with (
            tc.tile_pool(name="const", bufs=1) as cpool,
            tc.tile_pool(name="h", bufs=1) as hpool,
            tc.tile_pool(name="g", bufs=4) as gpool,
            tc.tile_pool(name="gtg", bufs=3) as gtgpool,
            tc.tile_pool(name="ks", bufs=2) as kspool,
            tc.tile_pool(name="ob", bufs=2) as obpool,
            tc.tile_pool(name="mx", bufs=1) as mxpool,
            tc.tile_pool(name="tp", bufs=4, space="PSUM") as tppool,
            tc.tile_pool(name="hp", bufs=1, space="PSUM") as hppool,
        ):
            cpk = cpool.tile([128, CPACK], f32)
            wc = cpool.tile([128, 2 * E], f32m)   # [d-chunk part, dchunk*256+e]
            ws = cpool.tile([128, 2 * E], f32m)
            if GATHER == "dma_gather":
                idx = cpool.tile([128, B_LOC * (NI // 16)], i16)
            else:
                idx = cpool.tile([128, 64], i32)
            nc.sync.dma_start(out=idx[:], in_=idx_d[:, :])
            nc.sync.dma_start(out=cpk[:], in_=cpk_d[:, :])
            nc.vector.tensor_copy(out=wc[:], in_=cpk[:, 0:512])
            nc.vector.tensor_copy(out=ws[:], in_=cpk[:, 512:1024])
            bias = cpk[:, 1024:1028]
            ident = cpk[:, 1028:1156]

            hts = [
                [hpool.tile([128, SW], f32, name=f"h{e}s{s}") for s in range(NSTAGE)]
                for e in range(2)
            ]
            mx = mxpool.tile([128, 2 * B_LOC], f32)

            def emit_init(s):
                for bb in range(SB):
                    b = SB * s + bb
                    g = gpool.tile([128, NIB * E], f32, name="g")
                    if GATHER == "dma_gather":
                        nc.gpsimd.dma_gather(
                            out_ap=g[:].rearrange("p (j e) -> p j e", e=E),
                            in_ap=encodes[HALF:, :],
                            idxs_ap=idx[:, b * (NI // 16) : (b + 1) * (NI // 16)],
                            num_idxs=NI,
                            num_idxs_reg=NI,
                            elem_size=E,
                            single_packet=False,
                        )
                    else:
                        for j in range(8):
                            nc.gpsimd.indirect_dma_start(
                                out=g[:, j * E : (j + 1) * E],
                                out_offset=None,
                                in_=encodes[:, :],
                                in_offset=bass.IndirectOffsetOnAxis(
                                    ap=idx[:, b * 8 + j : b * 8 + j + 1], axis=0
                                ),
                            )
                    gtgs = []
                    for gi in range(2):
                        gtg = gtgpool.tile([128, 4 * E], f32m, name="gtg")
                        gtgs.append(gtg)
                        for k in range(4):
                            j = gi * 4 + k
                            tp0 = tppool.tile([128, 128], f32, tag="tp", name="tp0")
                            tp1 = tppool.tile([128, 128], f32, tag="tp", name="tp1")
                            nc.tensor.transpose(
                                out=tp0[:], in_=g[:, j * E : j * E + 128],
                                identity=ident,
                            )
                            nc.tensor.transpose(
                                out=tp1[:], in_=g[:, j * E + 128 : (j + 1) * E],
                                identity=ident,
                            )
                            nc.vector.tensor_copy(
                                out=gtg[:, k * 128 : k * 128 + 128], in_=tp0[:]
                            )
                            nc.scalar.copy(
                                out=gtg[:, 512 + k * 128 : 512 + k * 128 + 128],
                                in_=tp1[:],
                            )
                    # weights-outer matmuls: 4 LDWEIGHTS, 8 matmuls per column
                    hp = {
                        (e, gi): hppool.tile(
                            [128, 512], f32, tag=f"hp{e}{gi}", name=f"hp{e}{gi}"
                        )
                        for e in range(2)
                        for gi in range(2)
                    }
                    for e in range(2):
                        for dc in range(2):
                            lhs = wc[:, 256 * dc + 128 * e : 256 * dc + 128 * e + 128]
                            for gi in range(2):
                                nc.tensor.matmul(
                                    out=hp[(e, gi)][:],
                                    lhsT=lhs,
                                    rhs=gtgs[gi][:, 512 * dc : 512 * dc + 512],
                                    start=(dc == 0),
                                    stop=(dc == 1),
                                )
                    for gi in range(2):
                        for e in range(2):
                            for p0, p1, leaf in _drain_runs_b(gi):
                                bcol = 1024 + (e if leaf else 2 + e)
                                nc.scalar.activation(
                                    out=hts[e][s][:, bb * P + p0 : bb * P + p1],
                                    in_=hp[(e, gi)][:, p0 - 512 * gi : p1 - 512 * gi],
                                    func=AF.Identity,
                                    bias=cpk[:, bcol : bcol + 1],
                                    scale=1.0,
                                )

            def emit_tree_out(s):
                hv = [
                    hts[e][s][:, :].rearrange("e (b q) -> e b q", b=SB)
                    for e in range(2)
                ]
                for d in range(DEPTH - 2, -1, -1):
                    p0 = 2**d - 1
                    n = 2**d
                    ks = [
                        kspool.tile([128, SB * n], f32m, tag=f"ks{e}", name=f"ks{e}")
                        for e in range(2)
                    ]
                    for e in range(2):
                        kid2 = hv[e][:, :, 2 * p0 + 1 : 2 * p0 + 1 + 2 * n].rearrange(
                            "e b (n two) -> e b n two", two=2
                        )
                        nc.vector.tensor_add(
                            out=ks[e][:].rearrange("e (b n) -> e b n", b=SB),
                            in0=kid2[:, :, :, 0],
                            in1=kid2[:, :, :, 1],
                        )
                    for e in range(2):
                        cs = hppool.tile(
                            [128, 512], f32, tag=f"hp{e}0", name=f"cs{e}"
                        )[:, : SB * n]
                        nc.tensor.matmul(
                            out=cs[:], lhsT=ws[:, 128 * e : 128 * e + 128],
                            rhs=ks[0][:, :],
                            start=True, stop=False,
                        )
                        nc.tensor.matmul(
                            out=cs[:],
                            lhsT=ws[:, 256 + 128 * e : 256 + 128 * e + 128],
                            rhs=ks[1][:, :],
                            start=False, stop=True,
                        )
                        nc.vector.tensor_add(
                            out=hv[e][:, :, p0 : p0 + n],
                            in0=hv[e][:, :, p0 : p0 + n],
                            in1=cs[:].rearrange("e (b n) -> e b n", b=SB),
                        )
                for e in range(2):
                    mxp = mxpool.tile([128, SB], f32, tag=f"mxp{e}", name=f"mxp{e}")
                    nc.vector.tensor_reduce(
                        out=mxp[:], in_=hv[e][:, :, :], axis=AX.X,
                        op=mybir.AluOpType.max,
                    )
                    nc.scalar.activation(
                        out=mx[:, e * B_LOC + SB * s : e * B_LOC + SB * (s + 1)],
                        in_=mxp[:],
                        func=AF.Relu,
                    )
                    ob = obpool.tile([128, SW], f32, name="ob")
                    nc.scalar.activation(out=ob[:], in_=hts[e][s][:, :], func=AF.Relu)
                    nc.sync.dma_start(out=out_nl[e, s, :, :], in_=ob[:])

            # one-stage-skew software pipeline
            for s in range(NSTAGE):
                emit_init(s)
                if s >= 1:
                    emit_tree_out(s - 1)
            emit_tree_out(NSTAGE - 1)

            nc.sync.dma_start(out=out_max[:, :], in_=mx[:])

    nc.compile()
    return nc


def kernel(**inputs):
    from concourse.bass_utils import run_bass_kernel_spmd

    encodes = np.ascontiguousarray(np.asarray(inputs["encodes"], dtype=np.float32))
    node_ids = np.asarray(inputs["node_ids"])
    wc = np.asarray(inputs["W_c_w"], dtype=np.float32).T  # [d, e]
    ws = np.asarray(inputs["W_sum_w"], dtype=np.float32).T
    bc = np.asarray(inputs["W_c_b"], dtype=np.float32)
    bs = np.asarray(inputs["W_sum_b"], dtype=np.float32)

    cpk = np.zeros((128, CPACK), np.float32)
    cpk[:, 0:256] = wc[0:128, :]
    cpk[:, 256:512] = wc[128:256, :]
    cpk[:, 512:768] = ws[0:128, :]
    cpk[:, 768:1024] = ws[128:256, :]
    cpk[:, 1024] = bc[0:128]
    cpk[:, 1025] = bc[128:256]
    cpk[:, 1026] = bc[0:128] + 2.0 * bs[0:128]
    cpk[:, 1027] = bc[128:256] + 2.0 * bs[128:256]
    cpk[:, 1028:1156] = np.eye(128, dtype=np.float32)

    in_maps = []
    for c in range(N_CORES):
        nid = np.asarray(node_ids[:, c * B_LOC : (c + 1) * B_LOC], dtype=np.int64)
        if GATHER == "dma_gather":
            cols = []
            for b in range(B_LOC):
                arr = np.zeros(NI, np.int64)
                arr[:P] = nid[:, b] - HALF
                arr[P:] = 0  # dummy: offset 0, keeps trailing idx >= 0
                wrapped = arr.astype(np.int16).reshape(NI // 16, 16).T
                cols.append(np.tile(wrapped, (8, 1)))
            idx = np.ascontiguousarray(np.concatenate(cols, axis=1))
        else:
            idx = np.zeros((128, 64), np.int32)
            for b in range(B_LOC):
                col = np.zeros(1024, np.int64)
                col[:P] = nid[:, b]
                idx[:, b * 8 : (b + 1) * 8] = col.reshape(8, 128).T
            idx = np.ascontiguousarray(idx)
        in_maps.append({"encodes": encodes, "idx": idx, "cpack": cpk})

    if _NC_CACHE[0] is None:
        _NC_CACHE[0] = _build_nc()
    nc = _NC_CACHE[0]

    res = run_bass_kernel_spmd(
        nc, in_maps, core_ids=list(range(N_CORES)), trace=TRACE[0]
    )
    LAST_RESULT[0] = res

    node_list = np.empty((P, B, E), np.float32)
    mx = np.empty((B, E), np.float32)
    for c in range(N_CORES):
        r = res.results[c]
        fm = r["out_nl"]  # [2, 4, 128, 2046]
        nl = (
            fm.reshape(2, NSTAGE, 128, SB, P)
            .transpose(1, 3, 4, 0, 2)
            .reshape(B_LOC, P, E)
        )
        node_list[:, c * B_LOC : (c + 1) * B_LOC, :] = nl.transpose(1, 0, 2)[POSTORDER]
        om = r["out_max"]
        mx[c * B_LOC : (c + 1) * B_LOC, 0:128] = om[:, 0:B_LOC].T
        mx[c * B_LOC : (c + 1) * B_LOC, 128:256] = om[:, B_LOC : 2 * B_LOC].T
    return node_list, mx


# revision 9
# speedup vs baseline: 1.5328x; 1.0677x over previous
"""Trainium2 Bass kernel for nn_BatchASTEncoder (batched AST / complete-binary-tree
GNN message passing).

Math (per batch column b):
    h[p] = W_c @ encodes[node_ids[p, b]] + b_c                    (1023 tree positions)
    for level d = 8..0:  h[parent] += W_sum @ (h[left] + h[right]) + 2*b_sum
    node_list = relu(h[POSTORDER]);  out2 = max_p node_list

Distribution: data-parallel over batch B=64 across 8 NeuronCores (8 columns
per core); encodes and the tiny weights are replicated.

Per-core dataflow (B_loc=8 batch columns, column index r = b*1023 + p):
  - h kept feature-major in SBUF: per (e-chunk, stage) tiles [128, 2046]
    (stage = 2 batch columns). Feature-major makes tree pair-sums strided
    free-dim vector ops and matmuls layout-preserving.
  - Software-pipelined stages with a one-stage skew in emission order so each
    engine's FIFO interleaves stage s's tree/output with stage s+1's init;
    the serial SWDGE gather descriptor generation is the pacing item.
  - Gather: one dma_gather per batch column. dma_gather indices are int16,
    so in_ap is based at the table midpoint and indices are signed offsets
    (idx - 32768); 17 trailing dummy indices keep the ext-isa kernel's
    trailing-negative truncation away from real rows.
  - Matmuls run in float32r (single-pass fp32, ~1e-4 relative error) instead
    of float32 (two-pass); producers of matmul operands write f32r tiles.
  - 2*W_sum_b is folded into the init bias of non-leaf positions.
  - Output is written feature-major; the host unshard step transposes to the
    reference row-major [P, B, E] layout and applies the postorder permute.
"""

import numpy as np

DEPTH = 10
P = 2**DEPTH - 1          # 1023
B = 64
E = 256
N_TOTAL = B * P           # 65472
HALF = 32768              # dma_gather int16 base offset
N_CORES = 8
B_LOC = B // N_CORES      # 8
R = B_LOC * P             # 8184 columns per core
NSTAGE = 4
SB = 2                    # batch columns per stage
SW = SB * P               # stage width: 2046
NI = 1040                 # dma_gather indices per call: 1023 real + 17 dummies
NIB = NI // 128 + 1       # dst j-blocks per call (9)
CPACK = 1156              # packed f32 consts: wc(512) ws(512) bias(4) ident(128)
GATHER = "dma_gather"     # or "indirect"
USE_F32R = True


def _postorder(p, out):
    if p >= P:
        return
    _postorder(2 * p + 1, out)
    _postorder(2 * p + 2, out)
    out.append(p)


_PO = []
_postorder(0, _PO)
POSTORDER = np.array(_PO, dtype=np.int32)

_NC_CACHE = [None]
LAST_RESULT = [None]
TRACE = [False]


def _drain_runs_b(grp_local):
    """Column runs for per-b matmul group (local p in [512g, 512(g+1)) cap 1023),
    split at the leaf boundary p=511. Yields (p0, p1, leaf)."""
    runs = []
    p = 512 * grp_local
    end = min(512 * (grp_local + 1), P)
    while p < end:
        leaf = p >= 511
        nxt = P if leaf else 511
        p1 = min(end, nxt)
        runs.append((p, p1, leaf))
        p = p1
    return runs


def _build_nc():
    import concourse.bacc as bacc
    import concourse.mybir as mybir
    import concourse.tile as tile
    from concourse import bass
    from concourse.masks import make_identity

    f32 = mybir.dt.float32
    f32m = mybir.dt.float32r if USE_F32R else f32
    i16 = mybir.dt.int16
    i32 = mybir.dt.int32
    AF = mybir.ActivationFunctionType
    AX = mybir.AxisListType

    nc = bacc.Bacc("TRN2", target_bir_lowering=False, debug=False)

    encodes = nc.dram_tensor("encodes", [N_TOTAL, E], f32, kind="ExternalInput")
    if GATHER == "dma_gather":
        idx_d = nc.dram_tensor(
            "idx", [128, B_LOC * (NI // 16)], i16, kind="ExternalInput"
        )
    else:
        idx_d = nc.dram_tensor("idx", [128, 64], i32, kind="ExternalInput")
    cpk_d = nc.dram_tensor("cpack", [128, CPACK], f32, kind="ExternalInput")
    out_nl = nc.dram_tensor("out_nl", [2, NSTAGE, 128, SW], f32, kind="ExternalOutput")
    out_max = nc.dram_tensor("out_max", [128, 2 * B_LOC], f32, kind="ExternalOutput")

    with tile.TileContext(nc) as tc:
        with (
            tc.tile_pool(name="const", bufs=1) as cpool,
            tc.tile_pool(name="h", bufs=1) as hpool,
            tc.tile_pool(name="g", bufs=4) as gpool,
            tc.tile_pool(name="gtg", bufs=3) as gtgpool,
            tc.tile_pool(name="ks", bufs=2) as kspool,
            tc.tile_pool(name="ob", bufs=2) as obpool,
            tc.tile_pool(name="mx", bufs=1) as mxpool,
            tc.tile_pool(name="tp", bufs=3, space="PSUM") as tppool,
            tc.tile_pool(name="hp", bufs=1, space="PSUM") as hppool,
            tc.tile_pool(name="dum", bufs=1, space="PSUM") as dumpool,
        ):
            cpk = cpool.tile([128, CPACK], f32)
            wc = cpool.tile([128, 2 * E], f32m)   # [d-chunk part, dchunk*256+e]
            ws = cpool.tile([128, 2 * E], f32m)
            if GATHER == "dma_gather":
                idx = cpool.tile([128, B_LOC * (NI // 16)], i16)
            else:
                idx = cpool.tile([128, 64], i32)
            nc.sync.dma_start(out=idx[:], in_=idx_d[:, :])
            nc.sync.dma_start(out=cpk[:], in_=cpk_d[:, :])
            nc.vector.tensor_copy(out=wc[:], in_=cpk[:, 0:512])
            nc.vector.tensor_copy(out=ws[:], in_=cpk[:, 512:1024])
            bias = cpk[:, 1024:1028]
            ident = cpk[:, 1028:1156]

            hts = [
                [hpool.tile([128, SW], f32, name=f"h{e}s{s}") for s in range(NSTAGE)]
                for e in range(2)
            ]
            mx = mxpool.tile([128, 2 * B_LOC], f32)

            def emit_init(s, bb):
                if True:
                    b = SB * s + bb
                    g = gpool.tile([128, NIB * E], f32, name="g")
                    if GATHER == "dma_gather":
                        nc.gpsimd.dma_gather(
                            out_ap=g[:].rearrange("p (j e) -> p j e", e=E),
                            in_ap=encodes[HALF:, :],
                            idxs_ap=idx[:, b * (NI // 16) : (b + 1) * (NI // 16)],
                            num_idxs=NI,
                            num_idxs_reg=NI,
                            elem_size=E,
                            single_packet=False,
                        )
                    else:
                        for j in range(8):
                            nc.gpsimd.indirect_dma_start(
                                out=g[:, j * E : (j + 1) * E],
                                out_offset=None,
                                in_=encodes[:, :],
                                in_offset=bass.IndirectOffsetOnAxis(
                                    ap=idx[:, b * 8 + j : b * 8 + j + 1], axis=0
                                ),
                            )
                    gtgs = []
                    for gi in range(2):
                        gtg = gtgpool.tile([128, 4 * E], f32m, name="gtg")
                        gtgs.append(gtg)
                        for k in range(4):
                            j = gi * 4 + k
                            tp0 = tppool.tile([128, 128], f32, tag="tp", name="tp0")
                            tp1 = tppool.tile([128, 128], f32, tag="tp", name="tp1")
                            nc.tensor.transpose(
                                out=tp0[:], in_=g[:, j * E : j * E + 128],
                                identity=ident,
                            )
                            nc.tensor.transpose(
                                out=tp1[:], in_=g[:, j * E + 128 : (j + 1) * E],
                                identity=ident,
                            )
                            nc.vector.tensor_copy(
                                out=gtg[:, k * 128 : k * 128 + 128], in_=tp0[:]
                            )
                            nc.scalar.copy(
                                out=gtg[:, 512 + k * 128 : 512 + k * 128 + 128],
                                in_=tp1[:],
                            )
                    # weights-outer matmuls: 4 LDWEIGHTS, 8 matmuls per column
                    hp = {
                        (e, gi): hppool.tile(
                            [128, 512], f32, tag=f"hp{e}{gi}", name=f"hp{e}{gi}"
                        )
                        for e in range(2)
                        for gi in range(2)
                    }
                    for e in range(2):
                        for dc in range(2):
                            lhs = wc[:, 256 * dc + 128 * e : 256 * dc + 128 * e + 128]
                            for gi in range(2):
                                nc.tensor.matmul(
                                    out=hp[(e, gi)][:],
                                    lhsT=lhs,
                                    rhs=gtgs[gi][:, 512 * dc : 512 * dc + 512],
                                    start=(dc == 0),
                                    stop=(dc == 1),
                                )
                    for gi in range(2):
                        for e in range(2):
                            for p0, p1, leaf in _drain_runs_b(gi):
                                bcol = 1024 + (e if leaf else 2 + e)
                                nc.scalar.activation(
                                    out=hts[e][s][:, bb * P + p0 : bb * P + p1],
                                    in_=hp[(e, gi)][:, p0 - 512 * gi : p1 - 512 * gi],
                                    func=AF.Identity,
                                    bias=cpk[:, bcol : bcol + 1],
                                    scale=1.0,
                                )

            def emit_leaf_out(s, bb):
                # leaf positions (p >= 511) are final right after init:
                # relu + store them early, and reduce their max
                for e in range(2):
                    ob = obpool.tile([128, 512], f32, tag="obl", name="obl")
                    nc.scalar.activation(
                        out=ob[:],
                        in_=hts[e][s][:, bb * P + 511 : bb * P + 1023],
                        func=AF.Relu,
                    )
                    nc.sync.dma_start(
                        out=out_nl[e, s, :, bb * P + 511 : bb * P + 1023],
                        in_=ob[:],
                    )
                # dummy matmul keeps the PE HAM window busy
                dmm = dumpool.tile([128, 512], f32, tag="dmm", name="dmm")
                nc.tensor.matmul(
                    out=dmm[:], lhsT=wc[:, 0:128], rhs=wc[:, :],
                    start=True, stop=True, skip_group_check=True,
                )

            def emit_tree_part(s, part):
                hv = [
                    hts[e][s][:, :].rearrange("e (b q) -> e b q", b=SB)
                    for e in range(2)
                ]
                levels = range(DEPTH - 2, 5, -1) if part == 0 else range(5, -1, -1)
                for d in levels:
                    p0 = 2**d - 1
                    n = 2**d
                    ks = [
                        kspool.tile([128, SB * n], f32m, tag=f"ks{e}", name=f"ks{e}")
                        for e in range(2)
                    ]
                    for e in range(2):
                        kid2 = hv[e][:, :, 2 * p0 + 1 : 2 * p0 + 1 + 2 * n].rearrange(
                            "e b (n two) -> e b n two", two=2
                        )
                        nc.vector.tensor_add(
                            out=ks[e][:].rearrange("e (b n) -> e b n", b=SB),
                            in0=kid2[:, :, :, 0],
                            in1=kid2[:, :, :, 1],
                        )
                    for e in range(2):
                        cs = hppool.tile(
                            [128, 512], f32, tag=f"hp{e}0", name=f"cs{e}"
                        )[:, : SB * n]
                        nc.tensor.matmul(
                            out=cs[:], lhsT=ws[:, 128 * e : 128 * e + 128],
                            rhs=ks[0][:, :],
                            start=True, stop=False,
                        )
                        nc.tensor.matmul(
                            out=cs[:],
                            lhsT=ws[:, 256 + 128 * e : 256 + 128 * e + 128],
                            rhs=ks[1][:, :],
                            start=False, stop=True,
                        )
                        nc.vector.tensor_add(
                            out=hv[e][:, :, p0 : p0 + n],
                            in0=hv[e][:, :, p0 : p0 + n],
                            in1=cs[:].rearrange("e (b n) -> e b n", b=SB),
                        )
                    dmm = dumpool.tile([128, 512], f32, tag="dmm", name="dmm")
                    nc.tensor.matmul(
                        out=dmm[:], lhsT=wc[:, 0:128], rhs=wc[:, :],
                        start=True, stop=True, skip_group_check=True,
                    )
                if part == 1:
                    # non-leaf positions now final: relu + store + stage max
                    for e in range(2):
                        mxp = mxpool.tile(
                            [128, SB], f32, tag=f"mxp{e}", name=f"mxp{e}"
                        )
                        nc.vector.tensor_reduce(
                            out=mxp[:], in_=hv[e][:, :, :], axis=AX.X,
                            op=mybir.AluOpType.max,
                        )
                        nc.scalar.activation(
                            out=mx[:, e * B_LOC + SB * s : e * B_LOC + SB * (s + 1)],
                            in_=mxp[:],
                            func=AF.Relu,
                        )
                        ob = obpool.tile([128, 2 * 511], f32, tag="obn", name="obn")
                        nc.scalar.activation(
                            out=ob[:].rearrange("p (b q) -> p b q", b=SB),
                            in_=hv[e][:, :, 0:511],
                            func=AF.Relu,
                        )
                        nc.sync.dma_start(
                            out=out_nl[e, s, :, :].rearrange(
                                "p (b q) -> p b q", b=SB
                            )[:, :, 0:511],
                            in_=ob[:].rearrange("p (b q) -> p b q", b=SB),
                        )

            # zip-emitted software pipeline (half-stage granularity)
            for s in range(NSTAGE):
                for bb in range(SB):
                    emit_init(s, bb)
                    emit_leaf_out(s, bb)
                    if s >= 1:
                        emit_tree_part(s - 1, bb)
            for bb in range(SB):
                emit_tree_part(NSTAGE - 1, bb)

            nc.sync.dma_start(out=out_max[:, :], in_=mx[:])

    nc.compile()
    return nc


def kernel(**inputs):
    from concourse.bass_utils import run_bass_kernel_spmd

    encodes = np.ascontiguousarray(np.asarray(inputs["encodes"], dtype=np.float32))
    node_ids = np.asarray(inputs["node_ids"])
    wc = np.asarray(inputs["W_c_w"], dtype=np.float32).T  # [d, e]
    ws = np.asarray(inputs["W_sum_w"], dtype=np.float32).T
    bc = np.asarray(inputs["W_c_b"], dtype=np.float32)
    bs = np.asarray(inputs["W_sum_b"], dtype=np.float32)

    cpk = np.zeros((128, CPACK), np.float32)
    cpk[:, 0:256] = wc[0:128, :]
    cpk[:, 256:512] = wc[128:256, :]
    cpk[:, 512:768] = ws[0:128, :]
    cpk[:, 768:1024] = ws[128:256, :]
    cpk[:, 1024] = bc[0:128]
    cpk[:, 1025] = bc[128:256]
    cpk[:, 1026] = bc[0:128] + 2.0 * bs[0:128]
    cpk[:, 1027] = bc[128:256] + 2.0 * bs[128:256]
    cpk[:, 1028:1156] = np.eye(128, dtype=np.float32)

    in_maps = []
    for c in range(N_CORES):
        nid = np.asarray(node_ids[:, c * B_LOC : (c + 1) * B_LOC], dtype=np.int64)
        if GATHER == "dma_gather":
            cols = []
            for b in range(B_LOC):
                arr = np.zeros(NI, np.int64)
                arr[:P] = nid[:, b] - HALF
                arr[P:] = 0  # dummy: offset 0, keeps trailing idx >= 0
                wrapped = arr.astype(np.int16).reshape(NI // 16, 16).T
                cols.append(np.tile(wrapped, (8, 1)))
            idx = np.ascontiguousarray(np.concatenate(cols, axis=1))
        else:
            idx = np.zeros((128, 64), np.int32)
            for b in range(B_LOC):
                col = np.zeros(1024, np.int64)
                col[:P] = nid[:, b]
                idx[:, b * 8 : (b + 1) * 8] = col.reshape(8, 128).T
            idx = np.ascontiguousarray(idx)
        in_maps.append({"encodes": encodes, "idx": idx, "cpack": cpk})

    if _NC_CACHE[0] is None:
        _NC_CACHE[0] = _build_nc()
    nc = _NC_CACHE[0]

    res = run_bass_kernel_spmd(
        nc, in_maps, core_ids=list(range(N_CORES)), trace=TRACE[0]
    )
    LAST_RESULT[0] = res

    node_list = np.empty((P, B, E), np.float32)
    mx = np.empty((B, E), np.float32)
    for c in range(N_CORES):
        r = res.results[c]
        fm = r["out_nl"]  # [2, 4, 128, 2046]
        nl = (
            fm.reshape(2, NSTAGE, 128, SB, P)
            .transpose(1, 3, 4, 0, 2)
            .reshape(B_LOC, P, E)
        )
        node_list[:, c * B_LOC : (c + 1) * B_LOC, :] = nl.transpose(1, 0, 2)[POSTORDER]
        om = r["out_max"]
        mx[c * B_LOC : (c + 1) * B_LOC, 0:128] = om[:, 0:B_LOC].T
        mx[c * B_LOC : (c + 1) * B_LOC, 128:256] = om[:, B_LOC : 2 * B_LOC].T
    return node_list, mx
